# revision 1
# baseline (speedup 1.0000x reference)
"""Trainium2 Bass kernel for nn_KnowledgeBasedLoss.

Math reduction (p = sigmoid(pred), power = 3):
  * every loss term is a mean over N = B*A = 134400 samples of products of
    p-derived per-channel values, so the whole kernel collapses to three
    channel-space statistics accumulated over N:
      q[n,c] = p^3 = exp(-3*softplus(-x)),  r[n,c] = (1-p)^3 = exp(-3*softplus(x))
      - Gram  QQ[c1,c2] = sum_n q[n,c1] q[n,c2]   (exclusion terms: (p_i p_j)^3 = q_i q_j)
      - Cross QR[c,j]   = sum_n q[n,c] r[n,j]     (conjunction: (p_c (1-p_j))^3)
      - Colsum SQ[c]    = sum_n q[n,c]            (disjunction power sums)
      - M[c]  = max_n x[n,c]                      (disjunction max, sigmoid applied later)
    QQ/QR/SQ come out of ONE PSUM-accumulated TensorE matmul series per core:
    lhsT = q[128,80], rhs = [q | r(ch 0..19) | ones] -> PSUM [80,101].
  * data-parallel over the batch axis: 8 cores x 16800 rows; host all-reduces
    the per-core [80,102] partials and applies the 1/3 roots + means.
"""
import os
import sys

sys.path.insert(0, "/opt/trn_rl_repo")

import numpy as np
from contextlib import ExitStack

import concourse.bass as bass
import concourse.tile as tile
from concourse import mybir
from concourse.masks import make_identity
from concourse.bass_utils import run_bass_kernel_spmd

dt = mybir.dt
AF = mybir.ActivationFunctionType

B, A, NCH = 16, 8400, 80
N_CORES = 8
ROWS_PER_CORE = B * A // N_CORES          # 16800
G = 33                                    # row-tiles per chunk
NCHUNK = 4
NT = G * NCHUNK                           # 132 row-tiles of 128 rows (16896, 96 pad)
PAD_ROWS = NT * 128 - ROWS_PER_CORE
PAD_VAL = -20.0                           # sigmoid(-20)^3 == 0 in fp32; never the max
N_TOTAL = float(B * A)

_CACHE = {}
LAST_RESULTS = None                       # test harness reads exec_time_ns from here


def _build_nc():
    nc = bass.Bass(trn_type="TRN2")
    x_d = nc.declare_dram_parameter("x", [128, NCHUNK, G, NCH], dt.float32, isOutput=False)
    out_d = nc.declare_dram_parameter("out", [80, 102], dt.float32, isOutput=True)

    with ExitStack() as ctx:
        tc = ctx.enter_context(tile.TileContext(nc))
        xp = ctx.enter_context(tc.tile_pool(name="xp", bufs=3))
        sp = ctx.enter_context(tc.tile_pool(name="sp", bufs=2))
        rp = ctx.enter_context(tc.tile_pool(name="rp", bufs=2))
        singles = ctx.enter_context(tc.tile_pool(name="singles", bufs=1))
        psum_p = ctx.enter_context(tc.tile_pool(name="psum", bufs=1, space="PSUM"))
        psum_t = ctx.enter_context(tc.tile_pool(name="psumt", bufs=1, space="PSUM"))

        ident = singles.tile([128, 128], dt.float32)
        make_identity(nc, ident)
        maxacc = singles.tile([128, NCH], dt.float32)
        nc.vector.memset(maxacc, -1e30)
        gram = psum_p.tile([80, 101], dt.float32)

        for chk in range(NCHUNK):
            xt = xp.tile([128, G, NCH], dt.float32, tag="xt")
            nc.sync.dma_start(out=xt, in_=x_d[:, chk])
            s1 = sp.tile([128, G, NCH], dt.float32, tag="s1")
            # q = sigmoid(x)^3 = exp(-3*softplus(-x))
            nc.scalar.activation(s1, xt, AF.Softplus, bias=0.0, scale=-1.0)
            rhs = rp.tile([128, G, 101], dt.bfloat16, tag="rhs")
            nc.scalar.activation(rhs[:, :, 0:80], s1, AF.Exp, bias=0.0, scale=-3.0)
            # r = (1-p)^3 = exp(-3*softplus(x)); only parent channels 0..19 are used
            s2 = sp.tile([128, G, 20], dt.float32, tag="s2")
            nc.scalar.activation(s2, xt[:, :, 0:20], AF.Softplus, bias=0.0, scale=1.0)
            nc.scalar.activation(rhs[:, :, 80:100], s2, AF.Exp, bias=0.0, scale=-3.0)
            nc.vector.memset(rhs[:, :, 100:101], 1.0)
            # per-channel running max of raw logits over this chunk's 33 tiles
            cmax = sp.tile([128, NCH], dt.float32, tag="cmax")
            nc.vector.reduce_max(cmax, xt.rearrange("p t c -> p c t"),
                                 axis=mybir.AxisListType.X)
            nc.vector.tensor_max(maxacc, maxacc, cmax)
            for t in range(G):
                g = chk * G + t
                nc.tensor.matmul(gram, lhsT=rhs[:, t, 0:80], rhs=rhs[:, t, :],
                                 start=(g == 0), stop=(g == NT - 1))

        outb = singles.tile([80, 102], dt.float32)
        nc.scalar.copy(outb[:, 0:101], gram)
        tp = psum_t.tile([80, 128], dt.float32)
        nc.tensor.transpose(tp, maxacc, ident)
        nc.vector.reduce_max(outb[:, 101:102], tp, axis=mybir.AxisListType.X)
        nc.sync.dma_start(out=out_d[:], in_=outb)
    return nc


def _get_nc():
    if "nc" not in _CACHE:
        _CACHE["nc"] = _build_nc()
    return _CACHE["nc"]


def _shard_host(pred_scores):
    """[16,8400,80] -> 8 per-core arrays [128, NCHUNK, G, 80] (row-tile major)."""
    maps = []
    per = B // N_CORES
    for c in range(N_CORES):
        rows = np.ascontiguousarray(
            pred_scores[c * per:(c + 1) * per].reshape(ROWS_PER_CORE, NCH))
        padded = np.concatenate(
            [rows, np.full((PAD_ROWS, NCH), PAD_VAL, dtype=np.float32)], axis=0)
        arr = padded.reshape(NT, 128, NCH).transpose(1, 0, 2)
        arr = np.ascontiguousarray(arr).reshape(128, NCHUNK, G, NCH)
        maps.append({"x": arr})
    return maps


def _finalize(gram_sum, xmax):
    """gram_sum [80,101] fp64 summed over cores; xmax [80] max over cores."""
    N = N_TOTAL
    p3 = 1.0 / 3.0
    pmax = 1.0 / (1.0 + np.exp(-xmax))
    QQ = gram_sum[:, 0:80]
    QR = gram_sum[:, 80:100]
    SQ = gram_sum[:, 100]

    ref_tgt = 10 + np.arange(70).reshape(10, 7)
    comp_tgt = 20 + np.arange(60).reshape(10, 6)

    m_ref = pmax[ref_tgt].max(axis=1)
    S_loss = np.mean((1.0 - m_ref) * (SQ[0:10] / N) ** p3)
    m_comp = pmax[comp_tgt].max(axis=1)
    C_loss = np.mean((1.0 - m_comp) * (SQ[10:20] / N) ** p3)

    pm = QQ[ref_tgt[:, :, None], ref_tgt[:, None, :]] / N
    v = np.maximum(pm, 0.0) ** p3
    SE_loss = np.mean((v.sum(axis=2) - np.diagonal(v, axis1=1, axis2=2)) / 6.0)
    pm = QQ[comp_tgt[:, :, None], comp_tgt[:, None, :]] / N
    v = np.maximum(pm, 0.0) ** p3
    CE_loss = np.mean((v.sum(axis=2) - np.diagonal(v, axis1=1, axis2=2)) / 5.0)

    ch = np.arange(10, 80)
    G_loss = np.mean(np.maximum(QR[ch, (ch - 10) // 7] / N, 0.0) ** p3)
    ch = np.arange(20, 80)
    D_loss = np.mean(np.maximum(QR[ch, 10 + (ch - 20) // 6] / N, 0.0) ** p3)

    return S_loss + C_loss + SE_loss + CE_loss + G_loss + D_loss


def kernel(pred_scores, target_scores=None, **_unused):
    global LAST_RESULTS
    nc = _get_nc()
    in_maps = _shard_host(np.asarray(pred_scores, dtype=np.float32))
    res = run_bass_kernel_spmd(nc, in_maps, core_ids=list(range(N_CORES)))
    LAST_RESULTS = res
    outs = [r["out"].astype(np.float64) for r in res.results]
    gram_sum = np.sum([o[:, 0:101] for o in outs], axis=0)
    xmax = np.max([o[:, 101] for o in outs], axis=0)
    total = _finalize(gram_sum, xmax)
    return np.array(total, dtype=np.float32)
